# revision 1
# baseline (speedup 1.0000x reference)
"""Trainium2 Bass kernel for a dense transformer block (B=128, T=256, C=384, H=6).

Sharding: data-parallel over batch across 8 NeuronCores (16 batches/core),
identical SPMD program per core, no collectives.

Per-core schedule: batches in pairs (free dim 512 in the big matmuls).
All matmuls in float32r (fp22-truncated fp32 at full PE speed for free
dim >= 256, fp32 PSUM accumulate).

This container's ACT piecewise-poly tables are broken (any table-based
activation crashes the device), so all transcendentals are built from
table-free DVE ops:
  - exp: Schraudolph bit-trick + quadratic mantissa correction (~3.5e-3 rel)
  - rsqrt (layernorm): sqrt bit-trick + 2 Heron steps + native reciprocal
  - relu: tensor_scalar (add bias, max 0)
Softmax skips max-subtraction (scores are O(1) here); the causal mask is
multiplicative post-exp; row-sum fuses into the mask multiply
(tensor_tensor_reduce); 1/rowsum folds into the PE transpose of the
attention weights via a diagonal rhs operand.
"""

import numpy as np

import concourse.bass as bass
import concourse.mybir as mybir
from concourse import bacc
from concourse.tile import TileContext
from contextlib import ExitStack

B, T, C = 128, 256, 384
H, D = 6, 64
FF = 4 * C
NCORES = 8
BL = B // NCORES  # 16
NPAIR = BL // 2  # 8
KC = C // 128  # 3
KH = FF // 128  # 12
EPS = 1e-5
F32 = mybir.dt.float32
F32R = mybir.dt.float32r
I32 = mybir.dt.int32
ALU = mybir.AluOpType

# exp = Schraudolph + quadratic mantissa correction (validated on HW: 3.5e-3)
EXP_S = float(2**23 / np.log(2.0))
EXP_B = float(127 * 2**23)
_C2, _C1, _C0 = 0.23374667, -0.2270202, 0.99663616
_ALPHA = _C1 / (2 * _C2)
_BETA = _C0 / _C2 - _ALPHA * _ALPHA
U_SCALE = float(np.sqrt(_C2) / 2**23)
U_BIAS = float(_ALPHA * np.sqrt(_C2))
E_BIAS = float(_BETA * _C2)
SQRT_MAGIC = 0x1FBD1DF5


def build_program(use_g1, use_b1ln, use_g2, use_b2ln, use_bp, use_b1, use_b2):
    nc = bacc.Bacc(None)
    x = nc.declare_dram_parameter("x", [BL, T, C], F32, isOutput=False)
    wq = nc.declare_dram_parameter("wq", [C, C], F32R, isOutput=False)
    wk = nc.declare_dram_parameter("wk", [C, C], F32R, isOutput=False)
    wv = nc.declare_dram_parameter("wv", [C, C], F32R, isOutput=False)
    wp = nc.declare_dram_parameter("wp", [C, C], F32R, isOutput=False)
    w1 = nc.declare_dram_parameter("w1", [C, FF], F32R, isOutput=False)
    w2 = nc.declare_dram_parameter("w2", [FF, C], F32R, isOutput=False)
    g1 = nc.declare_dram_parameter("g1", [128, C], F32, isOutput=False)
    b1ln = nc.declare_dram_parameter("b1ln", [128, C], F32, isOutput=False)
    g2 = nc.declare_dram_parameter("g2", [128, C], F32, isOutput=False)
    b2ln = nc.declare_dram_parameter("b2ln", [128, C], F32, isOutput=False)
    bpb = nc.declare_dram_parameter("bpb", [128, C], F32, isOutput=False)
    b2b = nc.declare_dram_parameter("b2b", [128, C], F32, isOutput=False)
    b1c = nc.declare_dram_parameter("b1c", [128, KH], F32, isOutput=False)
    m0 = nc.declare_dram_parameter("m0", [128, T], F32, isOutput=False)
    m1m = nc.declare_dram_parameter("m1m", [128, T], F32, isOutput=False)
    ident = nc.declare_dram_parameter("ident", [128, 128], F32, isOutput=False)
    out = nc.declare_dram_parameter("out", [BL, T, C], F32, isOutput=True)

    with TileContext(nc) as tc, ExitStack() as ctx:
        wts = ctx.enter_context(tc.tile_pool(name="wts", bufs=1))
        sb = ctx.enter_context(tc.tile_pool(name="sb", bufs=1))
        st = ctx.enter_context(tc.tile_pool(name="st", bufs=4))
        tr = ctx.enter_context(tc.tile_pool(name="tr", bufs=4))
        ps = ctx.enter_context(tc.tile_pool(name="ps", bufs=4, space="PSUM"))
        psy = ctx.enter_context(tc.tile_pool(name="psy", bufs=1, space="PSUM"))

        def load_chunks(dram, n, width, tagp):
            tiles = []
            for k in range(n):
                t_ = wts.tile(
                    [128, width], F32R, name=f"{tagp}{k}", tag=f"{tagp}{k}"
                )
                nc.sync.dma_start(out=t_, in_=dram[k * 128 : (k + 1) * 128, :])
                tiles.append(t_)
            return tiles

        wq_sb = load_chunks(wq, KC, C, "wq")
        wk_sb = load_chunks(wk, KC, C, "wk")
        wv_sb = load_chunks(wv, KC, C, "wv")
        wp_sb = load_chunks(wp, KC, C, "wp")
        w1_sb = load_chunks(w1, KC, FF, "w1")
        w2_sb = load_chunks(w2, KH, C, "w2")

        def load_one(dram, shape, tag):
            t_ = wts.tile(shape, F32, name=tag, tag=tag)
            nc.sync.dma_start(out=t_, in_=dram[:, :])
            return t_

        g1_sb = load_one(g1, [128, C], "g1") if use_g1 else None
        b1ln_sb = load_one(b1ln, [128, C], "b1ln") if use_b1ln else None
        g2_sb = load_one(g2, [128, C], "g2") if use_g2 else None
        b2ln_sb = load_one(b2ln, [128, C], "b2ln") if use_b2ln else None
        bpb_sb = load_one(bpb, [128, C], "bpb") if use_bp else None
        b2b_sb = load_one(b2b, [128, C], "b2b") if use_b2 else None
        b1c_sb = load_one(b1c, [128, KH], "b1c") if use_b1 else None
        m0_sb = load_one(m0, [128, T], "m0")
        m1_sb = load_one(m1m, [128, T], "m1m")
        id_sb = load_one(ident, [128, 128], "ident")

        for p in range(NPAIR):
            bs = [2 * p, 2 * p, 2 * p + 1, 2 * p + 1]
            tch = [0, 1, 0, 1]

            def batched_rstd(mv8):
                """[128,8] interleaved (mean,var) x4 -> rstd4 [128,4]."""
                mv_v = mv8.rearrange("p (i two) -> p i two", two=2)
                var4 = mv_v[:, :, 1]
                vpe = st.tile([128, 4], F32, name="vpe", tag="vpe")
                nc.vector.tensor_scalar(
                    out=vpe, in0=var4, scalar1=EPS, scalar2=None, op0=ALU.add
                )
                s0h = st.tile([128, 4], I32, name="s0h", tag="s0h")
                nc.vector.tensor_scalar(
                    out=s0h, in0=vpe.bitcast(I32), scalar1=1, scalar2=None,
                    op0=ALU.logical_shift_right,
                )
                s0i = st.tile([128, 4], I32, name="s0i", tag="s0i")
                nc.vector.tensor_scalar(
                    out=s0i, in0=s0h, scalar1=SQRT_MAGIC, scalar2=None,
                    op0=ALU.add,
                )
                cur = s0i.bitcast(F32)
                for hi in range(2):
                    r_ = st.tile([128, 4], F32, name=f"hr{hi}", tag=f"hr{hi}")
                    nc.vector.reciprocal(r_, cur)
                    t_ = st.tile([128, 4], F32, name=f"ht{hi}", tag=f"ht{hi}")
                    nc.vector.tensor_mul(t_, vpe, r_)
                    s_ = st.tile([128, 4], F32, name=f"hs{hi}", tag=f"hs{hi}")
                    nc.vector.tensor_add(s_, t_, cur)
                    sh = st.tile([128, 4], F32, name=f"hh{hi}", tag=f"hh{hi}")
                    nc.vector.tensor_scalar_mul(sh, s_, 0.5)
                    cur = sh
                rstd4 = st.tile([128, 4], F32, name="rstd4", tag="rstd4")
                nc.vector.reciprocal(rstd4, cur)
                return rstd4

            def layernorm4(dsts, srcs, g_sb, b_sb):
                mv8 = st.tile([128, 8], F32, name="mv8", tag="mv8")
                for i in range(4):
                    stats = st.tile([128, 6], F32, name="lst", tag="lst")
                    nc.vector.bn_stats(stats, srcs[i])
                    nc.vector.bn_aggr(mv8[:, 2 * i : 2 * i + 2], stats)
                rstd4 = batched_rstd(mv8)
                for i in range(4):
                    nc.vector.tensor_scalar(
                        out=dsts[i], in0=srcs[i],
                        scalar1=mv8[:, 2 * i : 2 * i + 1],
                        scalar2=rstd4[:, i : i + 1],
                        op0=ALU.subtract, op1=ALU.mult,
                    )
                    if g_sb is not None:
                        nc.vector.tensor_mul(dsts[i], dsts[i], g_sb)
                    if b_sb is not None:
                        nc.vector.tensor_add(dsts[i], dsts[i], b_sb)

            def transpose_into(dstT, src, i):
                for c in range(KC):
                    pt = ps.tile([128, 128], F32, name="pa", tag="pa")
                    nc.tensor.transpose(
                        pt, src[:, c * 128 : (c + 1) * 128], id_sb
                    )
                    nc.scalar.copy(dstT[c][:, i * 128 : (i + 1) * 128], pt)

            # ---- stage 1: load x, LN1, transpose -> hT ----
            xt = [
                sb.tile([128, C], F32, name=f"xt{i}", tag=f"xt{i}", bufs=2)
                for i in range(4)
            ]
            for i in range(4):
                nc.sync.dma_start(
                    out=xt[i],
                    in_=x[bs[i], tch[i] * 128 : (tch[i] + 1) * 128, :],
                )
            hT = [
                sb.tile([128, 2 * T], F32R, name=f"hT{c}", tag=f"hT{c}", bufs=2)
                for c in range(KC)
            ]
            ht_ = [
                sb.tile([128, C], F32, name=f"h{i}", tag=f"h{i}")
                for i in range(4)
            ]
            layernorm4(ht_, xt, g1_sb, b1ln_sb)
            for i in range(4):
                transpose_into(hT, ht_[i], i)

            # ---- stage 2: q^T, k^T (C-major), v (token-major) ----
            qT = [
                sb.tile([128, 2 * T], F32R, name=f"qT{m}", tag=f"qT{m}")
                for m in range(KC)
            ]
            kT = [
                sb.tile([128, 2 * T], F32R, name=f"kT{m}", tag=f"kT{m}")
                for m in range(KC)
            ]
            for m in range(KC):
                pq = ps.tile([128, 2 * T], F32, name="pa", tag="pa")
                for k in range(KC):
                    nc.tensor.matmul(
                        pq, wq_sb[k][:, m * 128 : (m + 1) * 128], hT[k],
                        start=(k == 0), stop=(k == KC - 1),
                    )
                nc.scalar.copy(qT[m], pq)
                pk = ps.tile([128, 2 * T], F32, name="pa", tag="pa")
                for k in range(KC):
                    nc.tensor.matmul(
                        pk, wk_sb[k][:, m * 128 : (m + 1) * 128], hT[k],
                        start=(k == 0), stop=(k == KC - 1),
                    )
                nc.scalar.copy(kT[m], pk)
            vt = [
                sb.tile([128, C], F32R, name=f"v{i}", tag=f"v{i}")
                for i in range(4)
            ]
            for i in range(4):
                pv = ps.tile([128, C], F32, name="pa", tag="pa")
                for k in range(KC):
                    nc.tensor.matmul(
                        pv, hT[k][:, i * 128 : (i + 1) * 128], wv_sb[k],
                        start=(k == 0), stop=(k == KC - 1),
                    )
                nc.scalar.copy(vt[i], pv)

            # ---- stage 3: attention (head pairs packed into PE col groups) ----
            acT = [
                sb.tile([128, 2 * T], F32R, name=f"acT{c}", tag=f"acT{c}")
                for c in range(KC)
            ]
            for ib in range(2):
                for ch in range(KC):
                    wTs = {}
                    for par in range(2):  # even/odd head of this chunk
                        hh = 2 * ch + par
                        off = par * 64
                        wTs[par] = [
                            tr.tile(
                                [128, T], F32R,
                                name=f"wT{sc}{par}", tag=f"wT{sc}{par}", bufs=2,
                            )
                            for sc in range(2)
                        ]
                        for tc_ in range(2):
                            w_ = 128 if tc_ == 0 else T
                            pS = ps.tile([128, T], F32, name="pa", tag="pa")
                            lhs = qT[ch][
                                off : off + 64,
                                ib * T + tc_ * 128 : ib * T + (tc_ + 1) * 128,
                            ]
                            rhs = kT[ch][off : off + 64, ib * T : (ib + 1) * T]
                            nc.tensor.matmul(pS, lhs, rhs, start=True, stop=True)
                            # exp via Schraudolph + quadratic correction (DVE)
                            it = tr.tile([128, T], I32, name="eit", tag="eit")
                            nc.vector.tensor_scalar(
                                out=it[:, 0:w_], in0=pS[:, 0:w_], scalar1=EXP_S,
                                scalar2=EXP_B, op0=ALU.mult, op1=ALU.add,
                            )
                            mt = tr.tile([128, T], I32, name="emt", tag="emt")
                            nc.vector.tensor_scalar(
                                out=mt[:, 0:w_], in0=it[:, 0:w_],
                                scalar1=0x7FFFFF, scalar2=None,
                                op0=ALU.bitwise_and,
                            )
                            ut = tr.tile([128, T], F32, name="eut", tag="eut")
                            nc.vector.tensor_scalar(
                                out=ut[:, 0:w_], in0=mt[:, 0:w_], scalar1=U_SCALE,
                                scalar2=U_BIAS, op0=ALU.mult, op1=ALU.add,
                            )
                            u2 = tr.tile([128, T], F32, name="eu2", tag="eu2")
                            nc.vector.tensor_mul(
                                u2[:, 0:w_], ut[:, 0:w_], ut[:, 0:w_]
                            )
                            eS = tr.tile([128, T], F32, name="eS", tag="eS")
                            nc.vector.scalar_tensor_tensor(
                                out=eS[:, 0:w_], in0=u2[:, 0:w_], scalar=E_BIAS,
                                in1=it.bitcast(F32)[:, 0:w_],
                                op0=ALU.add, op1=ALU.mult,
                            )
                            wU = tr.tile([128, T], F32, name="wU", tag="wU")
                            rsum = st.tile([128, 1], F32, name="rsum", tag="rsum")
                            nc.vector.scalar_tensor_tensor(
                                out=wU[:, 0:w_], in0=eS[:, 0:w_], scalar=1.0,
                                in1=(m0_sb if tc_ == 0 else m1_sb)[:, 0:w_],
                                op0=ALU.mult, op1=ALU.mult, accum_out=rsum,
                            )
                            rr = st.tile([128, 1], F32, name="rr", tag="rr")
                            nc.vector.reciprocal(rr, rsum)
                            wN = tr.tile([128, T], F32, name="wN", tag="wN")
                            nc.vector.tensor_scalar_mul(
                                wN[:, 0:w_], wU[:, 0:w_], rr
                            )
                            for sc in range(2 if tc_ else 1):
                                pt = ps.tile([128, 128], F32, name="pa", tag="pa")
                                nc.tensor.transpose(
                                    pt, wN[:, sc * 128 : (sc + 1) * 128], id_sb
                                )
                                nc.scalar.copy(
                                    wTs[par][sc][:, tc_ * 128 : (tc_ + 1) * 128],
                                    pt,
                                )
                    for par in range(2):
                        hh = 2 * ch + par
                        o_ = par * 64
                        pA = ps.tile([64, T], F32, name="pa", tag="pa")
                        nc.tensor.matmul(
                            pA, vt[ib * 2][:, hh * 64 : (hh + 1) * 64],
                            wTs[par][0], start=True, stop=False,
                        )
                        nc.tensor.matmul(
                            pA[:, 128:T],
                            vt[ib * 2 + 1][:, hh * 64 : (hh + 1) * 64],
                            wTs[par][1][:, 128:T], start=False, stop=True,
                        )
                        nc.scalar.copy(
                            acT[ch][o_ : o_ + 64, ib * T : (ib + 1) * T], pA
                        )

            # ---- stage 4: proj + residual -> y ----
            yt = [
                sb.tile([128, C], F32, name=f"y{i}", tag=f"y{i}")
                for i in range(4)
            ]
            for i in range(4):
                pP = ps.tile([128, C], F32, name="pa", tag="pa")
                for k in range(KC):
                    nc.tensor.matmul(
                        pP, acT[k][:, i * 128 : (i + 1) * 128], wp_sb[k],
                        start=(k == 0), stop=(k == KC - 1),
                    )
                nc.vector.tensor_add(yt[i], pP, xt[i])
                if bpb_sb is not None:
                    nc.vector.tensor_add(yt[i], yt[i], bpb_sb)

            # ---- stage 5: LN2 + transpose -> h2T ----
            h2T = [
                sb.tile([128, 2 * T], F32R, name=f"h2T{c}", tag=f"h2T{c}", bufs=2)
                for c in range(KC)
            ]
            h2_ = [
                sb.tile([128, C], F32, name=f"h2{i}", tag=f"h2{i}")
                for i in range(4)
            ]
            layernorm4(h2_, yt, g2_sb, b2ln_sb)
            for i in range(4):
                transpose_into(h2T, h2_[i], i)

            # ---- stage 6: MLP ----
            psY = [
                psy.tile([128, C], F32, name=f"psY{i}", tag=f"psY{i}")
                for i in range(4)
            ]
            for m in range(KH):
                pM = ps.tile([128, 2 * T], F32, name="pa", tag="pa")
                for k in range(KC):
                    nc.tensor.matmul(
                        pM, w1_sb[k][:, m * 128 : (m + 1) * 128], h2T[k],
                        start=(k == 0), stop=(k == KC - 1),
                    )
                m1r = tr.tile([128, 2 * T], F32R, name="m1r", tag="m1r")
                nc.vector.tensor_scalar(
                    out=m1r, in0=pM,
                    scalar1=(b1c_sb[:, m : m + 1] if use_b1 else 0.0),
                    scalar2=0.0, op0=ALU.add, op1=ALU.max,
                )
                for i in range(4):
                    nc.tensor.matmul(
                        psY[i], m1r[:, i * 128 : (i + 1) * 128], w2_sb[m],
                        start=(m == 0), stop=(m == KH - 1),
                    )

            # ---- stage 7: residual + store ----
            for i in range(4):
                ot = sb.tile([128, C], F32, name=f"ot{i}", tag=f"ot{i}", bufs=2)
                nc.vector.tensor_add(ot, psY[i], yt[i])
                if b2b_sb is not None:
                    nc.vector.tensor_add(ot, ot, b2b_sb)
                nc.sync.dma_start(
                    out=out[bs[i], tch[i] * 128 : (tch[i] + 1) * 128, :],
                    in_=ot,
                )
    nc.compile()
    return nc


def _host_prep(inputs):
    f = np.float32
    x = np.ascontiguousarray(inputs["x"], dtype=f)
    wq_full = np.ascontiguousarray(
        np.asarray(inputs["wq"], dtype=f).transpose(1, 0, 2).reshape(C, C)
        * (C ** -0.5)
    )
    wk_full = np.ascontiguousarray(
        np.asarray(inputs["wk"], dtype=f).transpose(1, 0, 2).reshape(C, C)
    )
    wv_full = np.ascontiguousarray(
        np.asarray(inputs["wv"], dtype=f).transpose(1, 0, 2).reshape(C, C)
    )
    wp = np.ascontiguousarray(inputs["w_proj"], dtype=f)
    w1 = np.ascontiguousarray(inputs["w1"], dtype=f)
    w2 = np.ascontiguousarray(inputs["w2"], dtype=f)
    tile128 = lambda v: np.ascontiguousarray(
        np.broadcast_to(np.asarray(v, dtype=f), (128, C))
    )
    g1 = tile128(inputs["ln1_g"])
    b1ln = tile128(inputs["ln1_b"])
    g2 = tile128(inputs["ln2_g"])
    b2ln = tile128(inputs["ln2_b"])
    bpb = tile128(inputs["b_proj"])
    b2b = tile128(inputs["b2"])
    b1c = np.ascontiguousarray(np.asarray(inputs["b1"], dtype=f).reshape(KH, 128).T)
    tril = np.tril(np.ones((128, 128), dtype=f))
    m0 = np.concatenate([tril, np.zeros((128, 128), dtype=f)], axis=1)
    m1m = np.concatenate([np.ones((128, 128), dtype=f), tril], axis=1)
    ident = np.eye(128, dtype=f)

    flags = (
        bool(not np.all(np.asarray(inputs["ln1_g"]) == 1.0)),
        bool(np.any(np.asarray(inputs["ln1_b"]))),
        bool(not np.all(np.asarray(inputs["ln2_g"]) == 1.0)),
        bool(np.any(np.asarray(inputs["ln2_b"]))),
        bool(np.any(np.asarray(inputs["b_proj"]))),
        bool(np.any(np.asarray(inputs["b1"]))),
        bool(np.any(np.asarray(inputs["b2"]))),
    )
    shared = dict(
        wq=wq_full, wk=wk_full, wv=wv_full, wp=wp, w1=w1, w2=w2,
        g1=g1, b1ln=b1ln, g2=g2, b2ln=b2ln, bpb=bpb, b2b=b2b, b1c=b1c,
        m0=m0, m1m=m1m, ident=ident,
    )
    in_maps = []
    for i in range(NCORES):
        m = dict(shared)
        m["x"] = np.ascontiguousarray(x[i * BL : (i + 1) * BL])
        in_maps.append(m)
    return in_maps, flags


_NC_CACHE = {}


def _get_program(flags):
    if flags not in _NC_CACHE:
        _NC_CACHE[flags] = build_program(*flags)
    return _NC_CACHE[flags]


def run(inputs, **spmd_kwargs):
    from concourse.bass_utils import run_bass_kernel_spmd

    in_maps, flags = _host_prep(inputs)
    nc = _get_program(flags)
    bkr = run_bass_kernel_spmd(nc, in_maps, list(range(NCORES)), **spmd_kwargs)
    outs = [bkr.results[i]["out"] for i in range(NCORES)]
    return np.concatenate(outs, axis=0).astype(np.float32), bkr


def kernel(**inputs):
    full, _ = run(inputs)
    return full



# revision 32
# speedup vs baseline: 1.7465x; 1.7465x over previous
"""Trainium2 Bass kernel for a dense transformer block (B=128, T=256, C=384, H=6).

Sharding: data-parallel over batch across 8 NeuronCores (16 batches/core),
identical SPMD program per core, no collectives.

v2 redesign (from v1 trace: DVE 56% busy, PE 46%, nearly serialized):
  - exp = raw Schraudolph (int bit-trick, one DVE op that also evacuates the
    scores PSUM). The constant part of Schraudolph's relative error cancels
    in softmax normalization; only the ~3% mantissa sawtooth survives, and
    softmax correlation shrinks it further.
  - causal mask folded into the Schraudolph bias operand: masked entries get
    a small bias so the resulting int bitcasts to ~1e-25 floats (exact-zero
    not needed).
  - 1/rowsum folded into the attention-weight transpose: regular matmul with
    a bf16 diag(1/rowsum) moving operand (1 cyc/row at any clock).
  - all transposes = regular matmuls against a bf16 identity (1 cyc/row).
  - rowsums via DVE tensor_reduce on the Schraudolph ints bitcast to f32.
  - layernorm apply on gpsimd (otherwise idle); stats stay on DVE.
  - PSUM evacuations batched through packed PSUM banks, copies on ACT.
  - kT / wT / acT / m1r / diag in bf16 (faster PE moving operands, less SBUF).
  - MLP down-projection i-outer so PSUM needs 1 rotating bank, not 4 held.
"""

import numpy as np

import concourse.bass as bass
import concourse.mybir as mybir
from concourse import bacc
from concourse.tile import TileContext
from contextlib import ExitStack

B, T, C = 128, 256, 384
H, D = 6, 64
FF = 4 * C
NCORES = 8
BL = B // NCORES  # 16
NPAIR = BL // 2  # 8
KC = C // 128  # 3
KH = FF // 128  # 12
EPS = 1e-5
F32 = mybir.dt.float32
F32R = mybir.dt.float32r
BF16 = mybir.dt.bfloat16
I32 = mybir.dt.int32
ALU = mybir.AluOpType
ACTF = mybir.ActivationFunctionType

EXP_S = float(2**23 / np.log(2.0))
EXP_B = float(127 * 2**23)
MASKB = 4.0e8  # masked scores -> it ~ 4e8 -> bitcast float ~1e-21 (safe to |s|<33)
SQRT_MAGIC = 0x1FBD1DF5
_STAGE = 99  # debug: truncate program after stage N (99 = full)


def build_program(use_g1, use_b1ln, use_g2, use_b2ln, use_bp, use_b1, use_b2):
    nc = bacc.Bacc(None)
    x = nc.declare_dram_parameter("x", [BL, T, C], F32, isOutput=False)
    wq = nc.declare_dram_parameter("wq", [C, C], F32R, isOutput=False)
    wk = nc.declare_dram_parameter("wk", [C, C], F32R, isOutput=False)
    wv = nc.declare_dram_parameter("wv", [C, C], F32R, isOutput=False)
    wp = nc.declare_dram_parameter("wp", [C, C], BF16, isOutput=False)
    w1 = nc.declare_dram_parameter("w1", [C, FF], F32R, isOutput=False)
    w2 = nc.declare_dram_parameter("w2", [FF, C], BF16, isOutput=False)
    g1 = nc.declare_dram_parameter("g1", [128, C], F32, isOutput=False)
    b1ln = nc.declare_dram_parameter("b1ln", [128, C], F32, isOutput=False)
    g2 = nc.declare_dram_parameter("g2", [128, C], F32, isOutput=False)
    b2ln = nc.declare_dram_parameter("b2ln", [128, C], F32, isOutput=False)
    bpb = nc.declare_dram_parameter("bpb", [128, C], F32, isOutput=False)
    b2b = nc.declare_dram_parameter("b2b", [128, C], F32, isOutput=False)
    b1c = nc.declare_dram_parameter("b1c", [128, KH], F32, isOutput=False)
    bias0 = nc.declare_dram_parameter("bias0", [128, 2 * 128], F32, isOutput=False)
    bias1 = nc.declare_dram_parameter("bias1", [128, 2 * T], F32, isOutput=False)
    ident = nc.declare_dram_parameter("ident", [128, 128], F32, isOutput=False)
    identb = nc.declare_dram_parameter("identb", [128, 128], BF16, isOutput=False)
    out = nc.declare_dram_parameter("out", [BL, T, C], F32, isOutput=True)

    with TileContext(nc) as tc, ExitStack() as ctx:
        wts = ctx.enter_context(tc.tile_pool(name="wts", bufs=1))
        sb = ctx.enter_context(tc.tile_pool(name="sb", bufs=1))
        st = ctx.enter_context(tc.tile_pool(name="st", bufs=4))
        tr = ctx.enter_context(tc.tile_pool(name="tr", bufs=4))
        ps = ctx.enter_context(tc.tile_pool(name="ps", bufs=6, space="PSUM"))
        psy = ctx.enter_context(tc.tile_pool(name="psy", bufs=2, space="PSUM"))

        def load_chunks(dram, n, width, tagp, dt=F32R):
            tiles = []
            for k in range(n):
                t_ = wts.tile(
                    [128, width], dt, name=f"{tagp}{k}", tag=f"{tagp}{k}"
                )
                nc.sync.dma_start(out=t_, in_=dram[k * 128 : (k + 1) * 128, :])
                tiles.append(t_)
            return tiles

        wq_sb = load_chunks(wq, KC, C, "wq")
        wk_sb = load_chunks(wk, KC, C, "wk")
        wv_sb = load_chunks(wv, KC, C, "wv")
        wp_sb = load_chunks(wp, KC, C, "wp", dt=BF16)
        w1_sb = load_chunks(w1, KC, FF, "w1")
        w2_sb = load_chunks(w2, KH, C, "w2", dt=BF16)

        def load_one(dram, shape, tag, dt=F32):
            t_ = wts.tile(shape, dt, name=tag, tag=tag)
            nc.sync.dma_start(out=t_, in_=dram[:, :])
            return t_

        g1_sb = load_one(g1, [128, C], "g1") if use_g1 else None
        b1ln_sb = load_one(b1ln, [128, C], "b1ln") if use_b1ln else None
        g2_sb = load_one(g2, [128, C], "g2") if use_g2 else None
        b2ln_sb = load_one(b2ln, [128, C], "b2ln") if use_b2ln else None
        bpb_sb = load_one(bpb, [128, C], "bpb") if use_bp else None
        b2b_sb = load_one(b2b, [128, C], "b2b") if use_b2 else None
        b1c_sb = load_one(b1c, [128, KH], "b1c") if use_b1 else None
        bias0_sb = load_one(bias0, [128, 2 * 128], "bias0")
        bias1_sb = load_one(bias1, [128, 2 * T], "bias1")
        id_f = load_one(ident, [128, 128], "identf")
        # bf16 identity: moving operand of the attention transpose-matmuls
        id_bf = load_one(identb, [128, 128], "identb", dt=BF16)

        for p in range(NPAIR):
            bs = [2 * p, 2 * p, 2 * p + 1, 2 * p + 1]
            tch = [0, 1, 0, 1]

            def batched_rstd(mv8):
                """[128,8] interleaved (mean,var) x4 -> rstd4 [128,4]."""
                mv_v = mv8.rearrange("p (i two) -> p i two", two=2)
                var4 = mv_v[:, :, 1]
                vpe = st.tile([128, 4], F32, name="vpe", tag="vpe")
                nc.vector.tensor_scalar(
                    out=vpe, in0=var4, scalar1=EPS, scalar2=None, op0=ALU.add
                )
                s0h = st.tile([128, 4], I32, name="s0h", tag="s0h")
                nc.vector.tensor_scalar(
                    out=s0h, in0=vpe.bitcast(I32), scalar1=1, scalar2=None,
                    op0=ALU.logical_shift_right,
                )
                s0i = st.tile([128, 4], I32, name="s0i", tag="s0i")
                nc.vector.tensor_scalar(
                    out=s0i, in0=s0h, scalar1=SQRT_MAGIC, scalar2=None,
                    op0=ALU.add,
                )
                cur = s0i.bitcast(F32)
                for hi in range(2):
                    r_ = st.tile([128, 4], F32, name=f"hr{hi}", tag=f"hr{hi}")
                    nc.vector.reciprocal(r_, cur)
                    t_ = st.tile([128, 4], F32, name=f"ht{hi}", tag=f"ht{hi}")
                    nc.vector.tensor_mul(t_, vpe, r_)
                    s_ = st.tile([128, 4], F32, name=f"hs{hi}", tag=f"hs{hi}")
                    nc.vector.tensor_add(s_, t_, cur)
                    sh = st.tile([128, 4], F32, name=f"hh{hi}", tag=f"hh{hi}")
                    nc.vector.tensor_scalar_mul(sh, s_, 0.5)
                    cur = sh
                rstd4 = st.tile([128, 4], F32, name="rstd4", tag="rstd4")
                nc.vector.reciprocal(rstd4, cur)
                return rstd4

            def layernorm4(dsts, srcs, g_sb, b_sb):
                mv8 = st.tile([128, 8], F32, name="mv8", tag="mv8")
                for i in range(4):
                    stats = st.tile([128, 6], F32, name="lst", tag="lst")
                    nc.vector.bn_stats(stats, srcs[i])
                    nc.vector.bn_aggr(mv8[:, 2 * i : 2 * i + 2], stats)
                rstd4 = batched_rstd(mv8)
                for i in range(4):
                    nc.vector.tensor_scalar(
                        out=dsts[i], in0=srcs[i],
                        scalar1=mv8[:, 2 * i : 2 * i + 1],
                        scalar2=rstd4[:, i : i + 1],
                        op0=ALU.subtract, op1=ALU.mult,
                    )
                    if g_sb is not None:
                        nc.vector.tensor_mul(dsts[i], dsts[i], g_sb)
                    if b_sb is not None:
                        nc.vector.tensor_add(dsts[i], dsts[i], b_sb)

            def transpose4_into(dstT, srcs):
                """4x [128,C] token-major -> dstT [128, KC*2T] C-major packed."""
                dst3 = dstT.rearrange("q (c w) -> q c w", c=KC)
                for i in range(4):
                    pt = ps.tile([128, C], F32, name="pa", tag="pa")
                    for c in range(KC):
                        nc.tensor.transpose(
                            pt[:, c * 128 : (c + 1) * 128],
                            srcs[i][:, c * 128 : (c + 1) * 128],
                            id_f,
                        )
                    nc.scalar.copy(
                        dst3[:, :, i * 128 : (i + 1) * 128],
                        pt.rearrange("q (c w) -> q c w", c=KC),
                    )

            # ---- stage 1: load x, LN1, transpose -> hT ----
            xt = [
                sb.tile([128, C], F32, name=f"xt{i}", tag=f"xt{i}", bufs=2)
                for i in range(4)
            ]
            for i in range(4):
                nc.sync.dma_start(
                    out=xt[i],
                    in_=x[bs[i], tch[i] * 128 : (tch[i] + 1) * 128, :],
                )
            hT = sb.tile(
                [128, KC * 2 * T], F32R, name="hT", tag="hT", bufs=2
            )
            ht_ = [
                sb.tile([128, C], F32, name=f"h{i}", tag=f"h{i}")
                for i in range(4)
            ]
            layernorm4(ht_, xt, g1_sb, b1ln_sb)
            transpose4_into(hT, ht_)
            if _STAGE < 20:
                for i in range(4):
                    ot = sb.tile([128, C], F32, name=f"ot{i}", tag=f"ot{i}", bufs=2)
                    nc.vector.tensor_add(ot, xt[i], xt[i])
                    nc.sync.dma_start(
                        out=out[bs[i], tch[i] * 128 : (tch[i] + 1) * 128, :],
                        in_=ot)
                continue

            # ---- stage 2: q^T (f32r), k^T (bf16) C-major; v token-major ----
            qT = [
                sb.tile([128, 2 * T], BF16, name=f"qT{m}", tag=f"qT{m}", bufs=2)
                for m in range(KC)
            ]
            kT = [
                sb.tile([128, 2 * T], BF16, name=f"kT{m}", tag=f"kT{m}", bufs=2)
                for m in range(KC)
            ]
            for m in range(KC):
                pq = ps.tile([128, 2 * T], F32, name="pa", tag="pa")
                for k in range(KC):
                    nc.tensor.matmul(
                        pq, wq_sb[k][:, m * 128 : (m + 1) * 128],
                        hT[:, k * 2 * T : (k + 1) * 2 * T],
                        start=(k == 0), stop=(k == KC - 1),
                    )
                nc.scalar.copy(qT[m], pq)
                pk = ps.tile([128, 2 * T], F32, name="pa", tag="pa")
                for k in range(KC):
                    nc.tensor.matmul(
                        pk, wk_sb[k][:, m * 128 : (m + 1) * 128],
                        hT[:, k * 2 * T : (k + 1) * 2 * T],
                        start=(k == 0), stop=(k == KC - 1),
                    )
                nc.scalar.copy(kT[m], pk)
            vt = [
                sb.tile([128, C], BF16, name=f"v{i}", tag=f"v{i}", bufs=2)
                for i in range(4)
            ]
            for i in range(4):
                pv = ps.tile([128, C], F32, name="pa", tag="pa")
                for k in range(KC):
                    nc.tensor.matmul(
                        pv, hT[:, k * 2 * T + i * 128 : k * 2 * T + (i + 1) * 128],
                        wv_sb[k],
                        start=(k == 0), stop=(k == KC - 1),
                    )
                nc.scalar.copy(vt[i], pv)

            if _STAGE < 30:
                for i in range(4):
                    ot = sb.tile([128, C], F32, name=f"ot{i}", tag=f"ot{i}", bufs=2)
                    nc.vector.tensor_add(ot, xt[i], xt[i])
                    nc.sync.dma_start(
                        out=out[bs[i], tch[i] * 128 : (tch[i] + 1) * 128, :],
                        in_=ot)
                continue
            # ---- stage 3: attention ----
            acT = [
                sb.tile([128, 2 * T], BF16, name=f"acT{c}", tag=f"acT{c}", bufs=2)
                for c in range(KC)
            ]
            for ib in range(2):
                for ch in range(KC):
                    tb = ib * T  # token base of batch ib in 2T-packed tiles
                    # NOTE: K=64 matmuls issued back-to-back into the SAME
                    # PSUM bank crash the device (concurrent sub-array
                    # drains collide); each head gets its own bank.
                    # scores tc0: queries 0..127, keys 0..127
                    pS0 = [ps.tile([128, 128], F32, name="pa", tag="pa")
                           for _ in range(2)]
                    for par in range(2):
                        o = par * 64
                        nc.tensor.matmul(
                            pS0[par],
                            qT[ch][o : o + 64, tb : tb + 128],
                            kT[ch][o : o + 64, tb : tb + 128],
                            start=True, stop=True,
                        )
                    if _STAGE < 31:
                        continue
                    it0 = tr.tile([128, 2 * 128], I32, name="it0", tag="it0",
                                  bufs=2)
                    for par in range(2):
                        nc.vector.scalar_tensor_tensor(
                            out=it0[:, par * 128 : (par + 1) * 128],
                            in0=pS0[par], scalar=EXP_S,
                            in1=bias0_sb[:, par * 128 : (par + 1) * 128],
                            op0=ALU.mult, op1=ALU.add,
                        )
                    # scores tc1: queries 128..255, keys 0..255
                    pS1 = [ps.tile([128, T], F32, name="pa", tag="pa")
                           for _ in range(2)]
                    for par in range(2):
                        o = par * 64
                        nc.tensor.matmul(
                            pS1[par],
                            qT[ch][o : o + 64, tb + 128 : tb + 2 * 128],
                            kT[ch][o : o + 64, tb : tb + T],
                            start=True, stop=True,
                        )
                    it1 = tr.tile([128, 2 * T], I32, name="it1", tag="it1",
                                  bufs=2)
                    for par in range(2):
                        nc.vector.scalar_tensor_tensor(
                            out=it1[:, par * T : (par + 1) * T],
                            in0=pS1[par], scalar=EXP_S,
                            in1=bias1_sb[:, par * T : (par + 1) * T],
                            op0=ALU.mult, op1=ALU.add,
                        )
                    if _STAGE < 33:
                        continue
                    # bf16 exp weights + rowsums per (tc, head) in one DVE op
                    rs4 = st.tile([128, 4], F32, name="rs4", tag="rs4")
                    it0f = it0.bitcast(F32)
                    it1f = it1.bitcast(F32)
                    yb0 = tr.tile([128, 2 * 128], BF16, name="yb0", tag="yb0",
                                  bufs=2)
                    yb1 = tr.tile([128, 2 * T], BF16, name="yb1", tag="yb1",
                                  bufs=2)
                    nc.vector.tensor_scalar(
                        out=yb0[:, 0:128], in0=it0f[:, 0:128], scalar1=1.0,
                        scalar2=None, op0=ALU.mult, op1=ALU.add,
                        accum_out=rs4[:, 0:1])
                    nc.vector.tensor_scalar(
                        out=yb0[:, 128:256], in0=it0f[:, 128:256], scalar1=1.0,
                        scalar2=None, op0=ALU.mult, op1=ALU.add,
                        accum_out=rs4[:, 1:2])
                    nc.vector.tensor_scalar(
                        out=yb1[:, 0:T], in0=it1f[:, 0:T], scalar1=1.0,
                        scalar2=None, op0=ALU.mult, op1=ALU.add,
                        accum_out=rs4[:, 2:3])
                    nc.vector.tensor_scalar(
                        out=yb1[:, T : 2 * T], in0=it1f[:, T : 2 * T],
                        scalar1=1.0,
                        scalar2=None, op0=ALU.mult, op1=ALU.add,
                        accum_out=rs4[:, 3:4])
                    if _STAGE < 34:
                        continue
                    rr4 = st.tile([128, 4], F32, name="rr4", tag="rr4")
                    nc.vector.reciprocal(rr4, rs4)
                    if _STAGE < 35:
                        continue
                    # diag(1/rowsum) in bf16 via ACT copy with per-row scale
                    dg = [
                        tr.tile([128, 128], BF16, name=f"dg{j}", tag=f"dg{j}",
                                bufs=2)
                        for j in range(4)
                    ]
                    for j in range(4):
                        nc.scalar.activation(
                            dg[j], id_bf, ACTF.Copy, bias=0.0,
                            scale=rr4[:, j : j + 1],
                        )
                    if _STAGE < 40:
                        continue
                    # transpose attention weights with folded normalization:
                    # regular all-bf16 matmul out = yb_slice^T @ diag(rr)
                    pA = ps.tile([128, 2 * T], F32, name="pa", tag="pa")
                    pB = ps.tile([128, T], F32, name="pa", tag="pa")
                    # layout A: [tc0-p0 | tc1-p0-k0 | tc0-p1 | tc1-p1-k0]
                    nc.tensor.matmul(pA[:, 0:128], yb0[:, 0:128], dg[0],
                                     start=True, stop=True)
                    nc.tensor.matmul(pA[:, 128:256], yb1[:, 0:128], dg[2],
                                     start=True, stop=True)
                    nc.tensor.matmul(pA[:, 256:384], yb0[:, 128:256], dg[1],
                                     start=True, stop=True)
                    nc.tensor.matmul(pA[:, 384:512], yb1[:, 256:384], dg[3],
                                     start=True, stop=True)
                    # layout B: [tc1-p0-k1 | tc1-p1-k1]
                    nc.tensor.matmul(pB[:, 0:128], yb1[:, 128:256], dg[2],
                                     start=True, stop=True)
                    nc.tensor.matmul(pB[:, 128:256], yb1[:, 384:512], dg[3],
                                     start=True, stop=True)
                    wT0 = tr.tile([128, 2 * T], BF16, name="wT0", tag="wT0",
                                  bufs=2)
                    wT1 = tr.tile([128, T], BF16, name="wT1", tag="wT1",
                                  bufs=2)
                    nc.scalar.copy(wT0, pA)
                    nc.scalar.copy(wT1, pB)
                    if _STAGE < 50:
                        continue
                    # apply: pC[par*64:(par+1)*64, q] = sum_k v[k, d] wT[k, q]
                    pC = ps.tile([128, T], F32, name="pa", tag="pa")
                    for par in range(2):
                        hh = 2 * ch + par
                        o = par * 64
                        nc.tensor.matmul(
                            pC[o : o + 64, 0:T],
                            vt[ib * 2][:, hh * 64 : (hh + 1) * 64],
                            wT0[:, par * T : (par + 1) * T],
                            start=True, stop=False,
                        )
                        nc.tensor.matmul(
                            pC[o : o + 64, 128:T],
                            vt[ib * 2 + 1][:, hh * 64 : (hh + 1) * 64],
                            wT1[:, par * 128 : (par + 1) * 128],
                            start=False, stop=True,
                        )
                    nc.scalar.copy(acT[ch][:, ib * T : (ib + 1) * T], pC)

            if _STAGE < 60:
                for i in range(4):
                    ot = sb.tile([128, C], F32, name=f"ot{i}", tag=f"ot{i}", bufs=2)
                    nc.vector.tensor_add(ot, xt[i], xt[i])
                    nc.sync.dma_start(
                        out=out[bs[i], tch[i] * 128 : (tch[i] + 1) * 128, :],
                        in_=ot)
                continue
            # ---- stage 4: proj + residual -> y ----
            yt = [
                sb.tile([128, C], F32, name=f"y{i}", tag=f"y{i}", bufs=2)
                for i in range(4)
            ]
            for i in range(4):
                pP = ps.tile([128, C], F32, name="pa", tag="pa")
                for k in range(KC):
                    nc.tensor.matmul(
                        pP, acT[k][:, i * 128 : (i + 1) * 128], wp_sb[k],
                        start=(k == 0), stop=(k == KC - 1),
                    )
                nc.vector.tensor_add(yt[i], pP, xt[i])
                if bpb_sb is not None:
                    nc.vector.tensor_add(yt[i], yt[i], bpb_sb)

            # ---- stage 5: LN2 + transpose -> h2T ----
            h2T = sb.tile(
                [128, KC * 2 * T], F32R, name="h2T", tag="h2T", bufs=2
            )
            h2_ = [
                sb.tile([128, C], F32, name=f"h2{i}", tag=f"h2{i}")
                for i in range(4)
            ]
            layernorm4(h2_, yt, g2_sb, b2ln_sb)
            transpose4_into(h2T, h2_)

            if _STAGE < 70:
                for i in range(4):
                    ot = sb.tile([128, C], F32, name=f"ot{i}", tag=f"ot{i}", bufs=2)
                    nc.vector.tensor_add(ot, yt[i], yt[i])
                    nc.sync.dma_start(
                        out=out[bs[i], tch[i] * 128 : (tch[i] + 1) * 128, :],
                        in_=ot)
                continue
            # ---- stage 6: MLP up + relu ----
            m1r = [
                sb.tile([128, 2 * T], BF16, name=f"m1r{m}", tag=f"m1r{m}")
                for m in range(KH)
            ]
            for m in range(KH):
                pM = ps.tile([128, 2 * T], F32, name="pa", tag="pa")
                for k in range(KC):
                    nc.tensor.matmul(
                        pM, w1_sb[k][:, m * 128 : (m + 1) * 128],
                        h2T[:, k * 2 * T : (k + 1) * 2 * T],
                        start=(k == 0), stop=(k == KC - 1),
                    )
                nc.vector.tensor_scalar(
                    out=m1r[m], in0=pM,
                    scalar1=(b1c_sb[:, m : m + 1] if use_b1 else 0.0),
                    scalar2=0.0, op0=ALU.add, op1=ALU.max,
                )

            # ---- stage 7: MLP down (i-outer) + residual + store ----
            for i in range(4):
                # padded to 512 so each psY slot is bank-aligned (mm out
                # must not cross a 2KB PSUM bank)
                pY = psy.tile([128, 512], F32, name="psY", tag="psY")
                for m in range(KH):
                    nc.tensor.matmul(
                        pY[:, 0:C], m1r[m][:, i * 128 : (i + 1) * 128], w2_sb[m],
                        start=(m == 0), stop=(m == KH - 1),
                    )
                ot = sb.tile([128, C], F32, name=f"ot{i}", tag=f"ot{i}", bufs=2)
                nc.vector.tensor_add(ot, pY[:, 0:C], yt[i])
                if b2b_sb is not None:
                    nc.vector.tensor_add(ot, ot, b2b_sb)
                nc.sync.dma_start(
                    out=out[bs[i], tch[i] * 128 : (tch[i] + 1) * 128, :],
                    in_=ot,
                )
    nc.compile()
    return nc


def _host_prep(inputs):
    f = np.float32
    x = np.ascontiguousarray(inputs["x"], dtype=f)
    wq_full = np.ascontiguousarray(
        np.asarray(inputs["wq"], dtype=f).transpose(1, 0, 2).reshape(C, C)
        * (C ** -0.5)
    )
    wk_full = np.ascontiguousarray(
        np.asarray(inputs["wk"], dtype=f).transpose(1, 0, 2).reshape(C, C)
    )
    wv_full = np.ascontiguousarray(
        np.asarray(inputs["wv"], dtype=f).transpose(1, 0, 2).reshape(C, C)
    )
    import ml_dtypes

    bf = ml_dtypes.bfloat16
    wp = np.ascontiguousarray(np.asarray(inputs["w_proj"], dtype=f).astype(bf))
    w1 = np.ascontiguousarray(inputs["w1"], dtype=f)
    w2 = np.ascontiguousarray(np.asarray(inputs["w2"], dtype=f).astype(bf))
    tile128 = lambda v: np.ascontiguousarray(
        np.broadcast_to(np.asarray(v, dtype=f), (128, C))
    )
    g1 = tile128(inputs["ln1_g"])
    b1ln = tile128(inputs["ln1_b"])
    g2 = tile128(inputs["ln2_g"])
    b2ln = tile128(inputs["ln2_b"])
    bpb = tile128(inputs["b_proj"])
    b2b = tile128(inputs["b2"])
    b1c = np.ascontiguousarray(np.asarray(inputs["b1"], dtype=f).reshape(KH, 128).T)
    tril = np.tril(np.ones((128, 128), dtype=bool))
    blk = np.where(tril, EXP_B, MASKB).astype(f)
    bias0 = np.ascontiguousarray(np.concatenate([blk, blk], axis=1))
    half = np.concatenate([np.full((128, 128), EXP_B, dtype=f), blk], axis=1)
    bias1 = np.ascontiguousarray(np.concatenate([half, half], axis=1))
    ident = np.eye(128, dtype=f)
    identb = np.eye(128, dtype=f).astype(bf)

    flags = (
        bool(not np.all(np.asarray(inputs["ln1_g"]) == 1.0)),
        bool(np.any(np.asarray(inputs["ln1_b"]))),
        bool(not np.all(np.asarray(inputs["ln2_g"]) == 1.0)),
        bool(np.any(np.asarray(inputs["ln2_b"]))),
        bool(np.any(np.asarray(inputs["b_proj"]))),
        bool(np.any(np.asarray(inputs["b1"]))),
        bool(np.any(np.asarray(inputs["b2"]))),
    )
    shared = dict(
        wq=wq_full, wk=wk_full, wv=wv_full, wp=wp, w1=w1, w2=w2,
        g1=g1, b1ln=b1ln, g2=g2, b2ln=b2ln, bpb=bpb, b2b=b2b, b1c=b1c,
        bias0=bias0, bias1=bias1, ident=ident, identb=identb,
    )
    in_maps = []
    for i in range(NCORES):
        m = dict(shared)
        m["x"] = np.ascontiguousarray(x[i * BL : (i + 1) * BL])
        in_maps.append(m)
    return in_maps, flags


_NC_CACHE = {}


def _get_program(flags):
    key = (flags, _STAGE)
    if key not in _NC_CACHE:
        _NC_CACHE[key] = build_program(*flags)
    return _NC_CACHE[key]


def run(inputs, **spmd_kwargs):
    from concourse.bass_utils import run_bass_kernel_spmd

    in_maps, flags = _host_prep(inputs)
    nc = _get_program(flags)
    bkr = run_bass_kernel_spmd(nc, in_maps, list(range(NCORES)), **spmd_kwargs)
    outs = [bkr.results[i]["out"] for i in range(NCORES)]
    return np.concatenate(outs, axis=0).astype(np.float32), bkr


def kernel(**inputs):
    full, _ = run(inputs)
    return full
